# revision 32
# baseline (speedup 1.0000x reference)
"""Trainium2 Bass kernel for nn_Attention_85564338471023.

Multi-head self-attention (B=16, N=1024 tokens, C=512, 8 heads x d=64) with
qkv projection, softmax attention, output projection and residual.

Sharding: pure data-parallel over batch -- 2 batch elements per NeuronCore,
no collectives.

Optimized v2: fp8e4m3 DoubleRow matmuls for the projections (contraction
256/instr) and for the scores (q/k repacked on-device into [32,2,N] DR
layout via SBUF-SBUF DMAs), and a fused [v_h | ones] fp8 stationary for the
attention*V matmul that produces the softmax denominator in the same pass
(rows 64..127 of the accumulator). exp runs with a -1.5 bias shift so fp8
ex stays within the TRN e4m3 +/-240 range; the shift cancels exactly in
res/s. Residual path (x + b_out + b_v@w_out) is exact f32 from the host.

Per-batch PE stream cycles: qk-proj 8192 + v-proj 4096 + scores 32768 +
AV 65536 + out-proj 16384 = 126976 (~2x less than the f32r baseline).
"""

import os
from contextlib import ExitStack

import numpy as np
import ml_dtypes

import concourse.bacc as bacc
import concourse.bass as bass
import concourse.tile as tile
from concourse import mybir
from concourse.bass_utils import run_bass_kernel_spmd  # noqa: F401 (fallback path)

N_CORES = 8
B, HH, WW, C = 16, 32, 32, 512
N = HH * WW            # 1024 tokens
NH, DH = 8, 64
SCALE = DH ** -0.5     # 0.125
BL = B // N_CORES      # 2 batch elements per core
P = 128
F32 = mybir.dt.float32
F32R = mybir.dt.float32r
BF16 = mybir.dt.bfloat16
F8 = mybir.dt.float8e4
DR = mybir.MatmulPerfMode.DoubleRow

EXP_SHIFT = 1.5        # ex = exp(s - 1.5): keeps fp8 ex <= ~135 << 240
# "fp8": fused [v|ones] fp8 M=128 stationary (denominator free)
# "fp8_127": fused M=127 (dodge fast-weight-load path) + row-63 fixup
# "bf16_127": fused M=127 bf16 stationary/ex (2-byte, !=128 cols: known-safe)
# "two": separate res/ones matmuls (baseline scheme), bf16 ex
# "two8": separate res/ones matmuls, fp8 ex/v (tests ScalarE fp8 out)
AV_MODE = os.environ.get("AV_MODE", "two")
DEBUG_XBASE = os.environ.get("DEBUG_XBASE", "0") == "1"


def build_program(repeat: int = 1) -> bass.Bass:
    nc = bacc.Bacc("TRN2", target_bir_lowering=False, debug=False)

    xt_d = nc.dram_tensor("xtdr", [BL, 2, P, 2, N], F8, kind="ExternalInput")
    rs_d = nc.dram_tensor("resid", [BL, N, C], F32, kind="ExternalInput")
    wqk_d = nc.dram_tensor("wqkdr", [2, P, 2, 1024], F8, kind="ExternalInput")
    bqk_d = nc.dram_tensor("bqk", [1024], F32, kind="ExternalInput")
    wv_d = nc.dram_tensor("wvdr", [2, P, 2, 512], F8, kind="ExternalInput")
    wout_d = nc.dram_tensor("wout", [C, 512], F32R, kind="ExternalInput")
    out_d = nc.dram_tensor("out", [BL, N, C], F32, kind="ExternalOutput")
    if DEBUG_XBASE:
        dbg_d = nc.dram_tensor("dbg", [64, 16], F32, kind="ExternalOutput")

    exdr = AV_MODE == "exdr"
    fused = AV_MODE in ("fp8", "fp8_127", "bf16_127", "fp8_65")
    fp8ex = AV_MODE in ("fp8", "fp8_127", "two8", "fp8_65", "exdr")
    av_dt = BF16 if AV_MODE == "bf16_127" else F8
    m_av = {"fp8_127": 127, "bf16_127": 127, "fp8_65": 65}.get(AV_MODE, 128)

    with tile.TileContext(nc) as tc, ExitStack() as ctx:
        consts = ctx.enter_context(tc.tile_pool(name="consts", bufs=1))
        wpool = ctx.enter_context(tc.tile_pool(name="w", bufs=1))
        xt_pool = ctx.enter_context(tc.tile_pool(name="xt", bufs=4))
        v8p = ctx.enter_context(tc.tile_pool(name="v8p", bufs=2))
        qk_pool = ctx.enter_context(tc.tile_pool(name="qk", bufs=8))
        qkdr_pool = ctx.enter_context(tc.tile_pool(name="qkdr", bufs=8))
        vo_pool = ctx.enter_context(tc.tile_pool(name="vo", bufs=8))
        ex_pool = ctx.enter_context(tc.tile_pool(name="ex", bufs=4))
        rt_pool = ctx.enter_context(tc.tile_pool(name="rt", bufs=8))
        misc = ctx.enter_context(tc.tile_pool(name="misc", bufs=2))
        # PSUM: 8 banks. psb: 2 x [128,1024] f32 = 4 banks (proj + scores).
        # psA: AV accumulators + out-proj: fused: 2 x [128,1024] = 4 banks;
        # two-mode: 4 x [128,512] = 4 banks.
        psb = ctx.enter_context(tc.tile_pool(name="psb", bufs=2, space="PSUM"))
        psA = ctx.enter_context(
            tc.tile_pool(name="psA", bufs=2 if fused else 4, space="PSUM"))

        bqk_sb = consts.tile([P, 8], F32, tag="bqk")
        nc.sync.dma_start(out=bqk_sb[:], in_=bqk_d.ap().rearrange("(t p) -> p t", p=P))
        expb = consts.tile([P, 1], F32, tag="expb")
        nc.vector.memset(expb[:], -EXP_SHIFT)
        if exdr:
            onesdr = consts.tile([P, 2, 64], F8, tag="onesdr")
            nc.vector.memset(onesdr[:], 1.0)
        elif not fused:
            ones = consts.tile([P, 64], F8 if fp8ex else BF16, tag="ones")
            nc.vector.memset(ones[:], 1.0)

        wqk_sb, wv_sb, wout_sb = [], [], []
        for kc in range(2):
            t = wpool.tile([P, 2, 1024], F8, tag=f"wqk{kc}")
            nc.sync.dma_start(out=t[:], in_=wqk_d.ap()[kc])
            wqk_sb.append(t)
            t = wpool.tile([P, 2, 512], F8, tag=f"wv{kc}")
            nc.sync.dma_start(out=t[:], in_=wv_d.ap()[kc])
            wv_sb.append(t)
        for kc in range(4):
            t = wpool.tile([P, 512], F32R, tag=f"wout{kc}")
            nc.sync.dma_start(out=t[:], in_=wout_d.ap()[kc * P:(kc + 1) * P, :])
            wout_sb.append(t)

        first_body = True
        for b in [b for _ in range(repeat) for b in range(BL)]:
            # ---- load xtdr (DR-packed channels) ----
            xt = []
            for kc in range(2):
                t = xt_pool.tile([P, 2, N], F8)
                nc.sync.dma_start(out=t[:], in_=xt_d.ap()[b, kc])
                xt.append(t)

            # ---- qk projection (DR): qkT[j-tile] = w_qk_tile.T @ xT ----
            # + bias/cast to fp8 + repack DMAs into [32,2,N] per head.
            kdr = [qkdr_pool.tile([P, 2, N], F8, tag="qkdr", name=f"kdr_{b}_{i}")
                   for i in range(2)]
            qdr = [qkdr_pool.tile([P, 2, N], F8, tag="qkdr", name=f"qdr_{b}_{i}")
                   for i in range(2)]
            for jt in range(8):
                ps = psb.tile([P, N], F32, tag="big")
                for ch in range(4):
                    for kc in range(2):
                        nc.tensor.matmul(
                            ps[:, ch * 256:(ch + 1) * 256],
                            wqk_sb[kc][:, :, jt * P:(jt + 1) * P],
                            xt[kc][:, :, ch * 256:(ch + 1) * 256],
                            start=(kc == 0), stop=(kc == 1),
                            perf_mode=DR,
                        )
                t = qk_pool.tile([P, N], F8)
                nc.vector.tensor_scalar(
                    out=t[:], in0=ps[:],
                    scalar1=bqk_sb[:, jt:jt + 1], scalar2=None,
                    op0=mybir.AluOpType.add,
                )
                # repack rows [e*64 + 32*sub + p] -> head slot rows, sub-half
                p4 = jt % 4
                dst = kdr if jt < 4 else qdr
                for e in range(2):
                    h = 2 * p4 + e
                    sl = 32 * (h % 4)
                    for sub in range(2):
                        nc.sync.dma_start(
                            out=dst[h // 4][sl:sl + 32, sub, :],
                            in_=t[e * 64 + 32 * sub: e * 64 + 32 * sub + 32, :],
                        )

            # ---- v projection (DR) + fused [v_h | ones] stationaries ----
            vo = []
            for yt in range(8):
                ps = psb.tile([P, N], F32, tag="big")
                for u in range(2):
                    for kc in range(2):
                        nc.tensor.matmul(
                            ps[:, u * 256:(u + 1) * 256],
                            xt[kc][:, :, yt * P:(yt + 1) * P],
                            wv_sb[kc][:, :, u * 256:(u + 1) * 256],
                            start=(kc == 0), stop=(kc == 1),
                            perf_mode=DR,
                        )
                if exdr:
                    # cast to fp8, then repack [j*64+r, u] -> [e*64+r, j, u]
                    # (duplicated across e-blocks to sit under each head's
                    # PE quadrant) for DoubleRow A*V.
                    v8 = v8p.tile([P, 512], F8)
                    nc.vector.tensor_copy(v8[:], ps[:, 0:512])
                    t = vo_pool.tile([P, 2, 512], F8)
                    for e in range(2):
                        for j in range(2):
                            nc.sync.dma_start(
                                out=t[e * 64:(e + 1) * 64, j, :],
                                in_=v8[j * 64:(j + 1) * 64, :],
                            )
                elif fused:
                    t = vo_pool.tile([P, 1024], av_dt)
                    tv = t[:].rearrange("p (h c) -> p h c", h=8)
                    nc.vector.tensor_copy(tv[:, :, 0:64], ps[:, 0:512])
                    nc.vector.memset(tv[:, :, 64:128], 1.0)
                else:
                    t = vo_pool.tile([P, 512], F8 if fp8ex else BF16)
                    nc.vector.tensor_copy(t[:], ps[:, 0:512])
                vo.append(t)

            # ---- attention, head pairs ----
            rt = []
            for p4 in range(4):
                rt_t = rt_pool.tile([P, N], F32R, tag="rt", name=f"rt_{b}_{p4}")
                if fused:
                    av = [psA.tile([P, N], F32, tag="av", name=f"av_{b}_{p4}_{e}")
                          for e in range(2)]
                else:
                    av = [psA.tile([P, 512], F32, tag="av", name=f"av_{b}_{p4}_{i}")
                          for i in range(2)]
                    s_ps = [psA.tile([P, 512], F32, tag="av", name=f"s_{b}_{p4}_{i}")
                            for i in range(2)]
                    if exdr:
                        # 256-col accumulation chunks share 2KB zero-region
                        # granules: use memset + start=False throughout.
                        for z in (*av, *s_ps):
                            nc.vector.memset(z[:], 0.0)

                def emit_av(yt, ex_pair):
                    first, last = yt == 0, yt == 7
                    for e in range(2):
                        h = 2 * p4 + e
                        if exdr:
                            esl = slice(e * 64, (e + 1) * 64)
                            for ch in range(4):
                                xs, c2 = divmod(ch, 2)
                                mov = ex_pair[esl, :, ch * 256:(ch + 1) * 256]
                                nc.tensor.matmul(
                                    av[xs][esl, c2 * 256:(c2 + 1) * 256],
                                    vo[yt][esl, :, h * 64:(h + 1) * 64],
                                    mov,
                                    start=False, stop=last,
                                    skip_group_check=True,
                                    perf_mode=DR,
                                    tile_position=(e * 64, e * 64),
                                )
                                nc.tensor.matmul(
                                    s_ps[xs][esl, c2 * 256:(c2 + 1) * 256],
                                    onesdr[esl, :, :],
                                    mov,
                                    start=False, stop=last,
                                    skip_group_check=True,
                                    perf_mode=DR,
                                    tile_position=(e * 64, e * 64),
                                )
                        elif fused:
                            for xs in range(2):
                                nc.tensor.matmul(
                                    av[e][0:m_av, xs * 512:(xs + 1) * 512],
                                    vo[yt][:, h * 128:h * 128 + m_av],
                                    ex_pair[e][:, xs * 512:(xs + 1) * 512],
                                    start=first, stop=last,
                                    skip_group_check=True,
                                )
                        else:
                            for xs in range(2):
                                exs = ex_pair[e][:, xs * 512:(xs + 1) * 512]
                                nc.tensor.matmul(
                                    av[xs][e * 64:(e + 1) * 64, :],
                                    vo[yt][:, h * 64:(h + 1) * 64],
                                    exs,
                                    start=first, stop=last,
                                    skip_group_check=True,
                                )
                                nc.tensor.matmul(
                                    s_ps[xs][e * 64:(e + 1) * 64, :],
                                    ones[:],
                                    exs,
                                    start=first, stop=last,
                                    skip_group_check=True,
                                )

                prev = None
                for yt in range(8):
                    if exdr:
                        # scores M=64 j-split: keys yt*128+j*64.. land in sc
                        # rows e*64..; exp writes j-halves of one fp8 ex tile
                        # [128, 2, 1024] that is DoubleRow-ready for A*V.
                        ex = ex_pool.tile([P, 2, N], F8)
                        for j in range(2):
                            sc = psb.tile([P, N], F32, tag="big")
                            kbase = yt * P + j * 64
                            for e in range(2):
                                h = 2 * p4 + e
                                sl = slice(32 * (h % 4), 32 * (h % 4) + 32)
                                for ch in range(4):
                                    nc.tensor.matmul(
                                        sc[e * 64:(e + 1) * 64,
                                           ch * 256:(ch + 1) * 256],
                                        kdr[h // 4][sl, :, kbase:kbase + 64],
                                        qdr[h // 4][sl, :, ch * 256:(ch + 1) * 256],
                                        start=True, stop=True,
                                        perf_mode=DR,
                                        tile_position=(32 * (h % 4), e * 64),
                                    )
                            nc.scalar.activation(
                                out=ex[:, j, :], in_=sc[:],
                                func=mybir.ActivationFunctionType.Exp,
                                scale=SCALE,
                                bias=expb[:],
                            )
                    else:
                        ex = []
                        for e in range(2):
                            h = 2 * p4 + e
                            sl = slice(32 * (h % 4), 32 * (h % 4) + 32)
                            sc = psb.tile([P, N], F32, tag="big")
                            for ch in range(4):
                                nc.tensor.matmul(
                                    sc[:, ch * 256:(ch + 1) * 256],
                                    kdr[h // 4][sl, :, yt * P:(yt + 1) * P],
                                    qdr[h // 4][sl, :, ch * 256:(ch + 1) * 256],
                                    start=True, stop=True,
                                    perf_mode=DR,
                                    tile_position=(32 * (h % 4), 0),
                                )
                            t = ex_pool.tile([P, N], av_dt if fused else (F8 if fp8ex else BF16))
                            nc.scalar.activation(
                                out=t[:], in_=sc[:],
                                func=mybir.ActivationFunctionType.Exp,
                                scale=SCALE,
                                bias=expb[:] if fp8ex else 0.0,
                            )
                            ex.append(t)
                    if prev is not None:
                        emit_av(*prev)
                    prev = (yt, ex)
                emit_av(*prev)

                if fused:
                    for e in range(2):
                        nsum = m_av - 64  # denominator copies in rows 64..m_av
                        rec = misc.tile([64, N], F32, tag="prc")
                        if nsum == 1:
                            rec1 = misc.tile([1, N], F32, tag="prc1")
                            nc.vector.reciprocal_approx_fast(
                                out=rec1[:], in_=av[e][64:65, :])
                            nc.gpsimd.partition_broadcast(rec[:], rec1[:])
                        else:
                            nc.vector.reciprocal_approx_fast(
                                out=rec[0:nsum, :], in_=av[e][64:m_av, :])
                            if nsum < 64:
                                # s-copies identical per column; refill rows
                                # 32:64 from 0:32 so row 63 is valid too
                                nc.vector.tensor_copy(rec[32:64, :], rec[0:32, :])
                        nc.vector.tensor_tensor(
                            out=rt_t[e * 64:(e + 1) * 64, :],
                            in0=av[e][0:64, :], in1=rec[:],
                            op=mybir.AluOpType.mult,
                        )
                else:
                    for xs in range(2):
                        rec = misc.tile([P, 512], F32, tag="prc")
                        nc.vector.reciprocal_approx_fast(
                            out=rec[:], in_=s_ps[xs][:])
                        nc.vector.tensor_tensor(
                            out=rt_t[:, xs * 512:(xs + 1) * 512],
                            in0=av[xs][:], in1=rec[:],
                            op=mybir.AluOpType.mult,
                        )
                rt.append(rt_t)

            # ---- output projection + residual ----
            for nt in range(8):
                ps = psA.tile([P, N] if fused else [P, 512], F32, tag="av")
                for p4 in range(4):
                    nc.tensor.matmul(
                        ps[:, 0:512],
                        rt[p4][:, nt * P:(nt + 1) * P],
                        wout_sb[p4][:],
                        start=(p4 == 0), stop=(p4 == 3),
                    )
                rs = misc.tile([P, 512], F32, tag="rs")
                nc.sync.dma_start(out=rs[:], in_=rs_d.ap()[b, nt * P:(nt + 1) * P, :])
                if DEBUG_XBASE and first_body and b == 0 and nt == 0:
                    dbg_sb = misc.tile([64, 16], F32, tag="dbg")
                    nc.vector.tensor_copy(dbg_sb[:], rs[64:128, 0:16])
                    nc.sync.dma_start(out=dbg_d.ap()[:], in_=dbg_sb[:])
                ob = misc.tile([P, 512], F32, tag="ob")
                nc.vector.tensor_tensor(
                    out=ob[:], in0=ps[:, 0:512], in1=rs[:], op=mybir.AluOpType.add,
                )
                nc.sync.dma_start(out=out_d.ap()[b, nt * P:(nt + 1) * P, :], in_=ob[:])
            first_body = False

    nc.compile()
    return nc


NP_F8 = ml_dtypes.float8_e4m3fn


def host_prep(ft, w_qkv, b_qkv, w_out, b_out):
    ft = np.asarray(ft, dtype=np.float32)
    w_qkv = np.asarray(w_qkv, dtype=np.float32)
    b_qkv = np.asarray(b_qkv, dtype=np.float32)
    w_out = np.asarray(w_out, dtype=np.float32)
    b_out = np.asarray(b_out, dtype=np.float32)

    x = ft.reshape(B, N, C)

    # pair-grouped column orders (k block then q block; v natural pairs)
    w_qk_re = np.empty((C, 1024), np.float32)
    b_qk_re = np.empty((1024,), np.float32)
    w_v_re = np.empty((C, 512), np.float32)
    for p in range(4):
        hA, hB = 2 * p, 2 * p + 1
        w_qk_re[:, p * 128:p * 128 + 64] = w_qkv[:, hA * 192 + 64:hA * 192 + 128]
        w_qk_re[:, p * 128 + 64:p * 128 + 128] = w_qkv[:, hB * 192 + 64:hB * 192 + 128]
        b_qk_re[p * 128:p * 128 + 64] = b_qkv[hA * 192 + 64:hA * 192 + 128]
        b_qk_re[p * 128 + 64:p * 128 + 128] = b_qkv[hB * 192 + 64:hB * 192 + 128]
        w_qk_re[:, 512 + p * 128:512 + p * 128 + 64] = w_qkv[:, hA * 192:hA * 192 + 64]
        w_qk_re[:, 512 + p * 128 + 64:512 + p * 128 + 128] = w_qkv[:, hB * 192:hB * 192 + 64]
        b_qk_re[512 + p * 128:512 + p * 128 + 64] = b_qkv[hA * 192:hA * 192 + 64]
        b_qk_re[512 + p * 128 + 64:512 + p * 128 + 128] = b_qkv[hB * 192:hB * 192 + 64]
        w_v_re[:, p * 128:p * 128 + 64] = w_qkv[:, hA * 192 + 128:hA * 192 + 192]
        w_v_re[:, p * 128 + 64:p * 128 + 128] = w_qkv[:, hB * 192 + 128:hB * 192 + 192]

    b_v_nat = np.empty((512,), np.float32)
    for h in range(NH):
        b_v_nat[h * 64:(h + 1) * 64] = b_qkv[h * 192 + 128:h * 192 + 192]
    resid = x + b_out[None, None, :] + (b_v_nat @ w_out)[None, None, :]
    resid = np.ascontiguousarray(resid, dtype=np.float32)

    # DR packs: channel c = kc*256 + sub*128 + p
    xT = x.transpose(0, 2, 1)                      # [B, C, N]
    xtdr = np.ascontiguousarray(
        xT.reshape(B, 2, 2, P, N).transpose(0, 1, 3, 2, 4)).astype(NP_F8)
    wqkdr = np.ascontiguousarray(
        w_qk_re.reshape(2, 2, P, 1024).transpose(0, 2, 1, 3)).astype(NP_F8)
    wvdr = np.ascontiguousarray(
        w_v_re.reshape(2, 2, P, 512).transpose(0, 2, 1, 3)).astype(NP_F8)
    return xtdr, resid, wqkdr, b_qk_re, wvdr, np.ascontiguousarray(w_out)


_NC_CACHE = {}


def get_program(repeat: int = 1) -> bass.Bass:
    if repeat not in _NC_CACHE:
        _NC_CACHE[repeat] = build_program(repeat)
    return _NC_CACHE[repeat]


def make_in_maps(ft, w_qkv, b_qkv, w_out, b_out):
    xtdr, resid, wqkdr, b_qk_re, wvdr, w_out_c = host_prep(
        ft, w_qkv, b_qkv, w_out, b_out)
    in_maps = []
    for core in range(N_CORES):
        sl = slice(core * BL, (core + 1) * BL)
        in_maps.append({
            "xtdr": np.ascontiguousarray(xtdr[sl]),
            "resid": np.ascontiguousarray(resid[sl]),
            "wqkdr": wqkdr,
            "bqk": b_qk_re,
            "wvdr": wvdr,
            "wout": w_out_c,
        })
    return in_maps


_RUNNER_CACHE = {}


def make_runner(repeat: int = 1):
    """Build (once) a persistent jitted executor for the bass program."""
    if repeat in _RUNNER_CACHE:
        return _RUNNER_CACHE[repeat]

    import jax
    from jax.experimental.shard_map import shard_map
    from jax.sharding import Mesh, PartitionSpec
    from concourse import mybir as _mb
    from concourse import bass2jax

    bass2jax.install_neuronx_cc_hook()
    nc = get_program(repeat)

    partition_name = nc.partition_id_tensor.name if nc.partition_id_tensor else None
    in_names, out_names, out_avals, zero_shapes = [], [], [], []
    for alloc in nc.m.functions[0].allocations:
        if not isinstance(alloc, _mb.MemoryLocationSet):
            continue
        name = alloc.memorylocations[0].name
        if alloc.kind == "ExternalInput":
            if name != partition_name:
                in_names.append(name)
        elif alloc.kind == "ExternalOutput":
            np_dt = _mb.dt.np(alloc.dtype)
            out_names.append(name)
            out_avals.append(jax.core.ShapedArray(tuple(alloc.tensor_shape), np_dt))
            zero_shapes.append((tuple(alloc.tensor_shape), np_dt))
    n_params = len(in_names)
    all_in_names = list(in_names) + list(out_names)
    if partition_name is not None:
        all_in_names.append(partition_name)

    def _body(*args):
        operands = list(args)
        if partition_name is not None:
            operands.append(bass2jax.partition_id_tensor())
        outs = bass2jax._bass_exec_p.bind(
            *operands,
            out_avals=tuple(out_avals),
            in_names=tuple(all_in_names),
            out_names=tuple(out_names),
            lowering_input_output_aliases=(),
            sim_require_finite=True,
            sim_require_nnan=True,
            nc=nc,
        )
        return tuple(outs)

    devices = jax.devices()[:N_CORES]
    mesh = Mesh(np.asarray(devices), ("core",))
    n_outs = len(out_names)
    sharded = jax.jit(
        shard_map(_body, mesh=mesh,
                  in_specs=(PartitionSpec("core"),) * (n_params + n_outs),
                  out_specs=(PartitionSpec("core"),) * n_outs,
                  check_rep=False),
        keep_unused=True,
    )

    def run(in_maps):
        concat_in = [
            np.concatenate([np.asarray(m[name]) for m in in_maps], axis=0)
            for name in in_names
        ]
        zeros = [np.zeros((N_CORES * s[0], *s[1:]), dt) for s, dt in zero_shapes]
        out_arrs = sharded(*concat_in, *zeros)
        return [
            {name: np.asarray(out_arrs[i]).reshape(N_CORES, *out_avals[i].shape)[c]
             for i, name in enumerate(out_names)}
            for c in range(N_CORES)
        ]

    run.sharded = sharded
    run.in_names = in_names
    run.zero_shapes = zero_shapes
    run.mesh = mesh
    _RUNNER_CACHE[repeat] = run
    return run


def kernel(ft, w_qkv, b_qkv, w_out, b_out):
    run = make_runner()
    in_maps = make_in_maps(ft, w_qkv, b_qkv, w_out, b_out)
    results = run(in_maps)
    out = np.concatenate([r["out"] for r in results], axis=0)
    return out.reshape(B, HH, WW, C).astype(np.float32)


# revision 36
# speedup vs baseline: 1.0274x; 1.0274x over previous
"""Trainium2 Bass kernel for nn_Attention_85564338471023.

Multi-head self-attention (B=16, N=1024 tokens, C=512, 8 heads x d=64) with
qkv projection, softmax attention, output projection and residual.

Sharding: pure data-parallel over batch -- 2 batch elements per NeuronCore,
no collectives. Host pre-transposes x (channels-on-partitions) and reorders
w_qkv columns so heads come in pairs that share 128-partition tiles.

Device algorithm per batch element (all matmuls float32r, full PE rate):
  qkT[j',n]  = w_qk_re.T @ xT      (j' pair-grouped: [kA|kB]x4 then [qA|qB]x4)
  v[y,u]     = x @ w_v_re          (natural token-major layout, pair-grouped)
  per head (row-group paired, 2 heads concurrent in PE array):
    scT[y,x] = kT.T @ qT           (scores TRANSPOSED: keys on partitions)
    ex       = exp(scT / 8)        (ScalarE, scale fused; max-sub skipped --
                                    scores are ~N(0,1), |s|<8 in practice)
    res[d,x] = v_h.T @ ex          (col-group paired; accumulated over y-tiles)
    s[x]     = ones(64).T @ ex     (denominator; the 64 ones-columns broadcast
                                    s across the 64 partitions of its head)
    rt       = res * recip(s)      (DVE reciprocal_approx_fast + multiply)
  out[x,c]   = rt.T @ w_out + resid   (resid = x + b_out + b_v@w_out, host)
"""

from contextlib import ExitStack

import numpy as np

import concourse.bacc as bacc
import concourse.bass as bass
import concourse.tile as tile
from concourse import mybir
from concourse.bass_utils import run_bass_kernel_spmd  # noqa: F401 (fallback path)

N_CORES = 8
B, HH, WW, C = 16, 32, 32, 512
N = HH * WW            # 1024 tokens
NH, DH = 8, 64
SCALE = DH ** -0.5     # 0.125
BL = B // N_CORES      # 2 batch elements per core
P = 128
F32 = mybir.dt.float32
F32R = mybir.dt.float32r

# matmul compute dtype: float32r streams 1 row/cycle at N>=256 (4x faster
# than plain fp32 on the PE) at ~tf32-ish precision -- projections + scores.
# The A*V matmul uses bf16 ex/v with M=64 stationaries: wider fused
# [v_h | ones] stationaries (M=65/127/128, any dtype) were observed to
# corrupt results on HW under interleaved PSUM accumulation.
MM_DT = F32R
AV_DT = mybir.dt.bfloat16


def _mm(ap):
    return ap


def build_program(repeat: int = 1) -> bass.Bass:
    nc = bacc.Bacc("TRN2", target_bir_lowering=False, debug=False)

    xT_d = nc.dram_tensor("xT", [BL, C, N], MM_DT, kind="ExternalInput")
    rs_d = nc.dram_tensor("resid", [BL, N, C], F32, kind="ExternalInput")
    wqk_d = nc.dram_tensor("wqk", [C, 1024], MM_DT, kind="ExternalInput")
    bqk_d = nc.dram_tensor("bqk", [1024], F32, kind="ExternalInput")
    wv_d = nc.dram_tensor("wv", [C, 512], MM_DT, kind="ExternalInput")
    wout_d = nc.dram_tensor("wout", [C, 512], MM_DT, kind="ExternalInput")
    out_d = nc.dram_tensor("out", [BL, N, C], F32, kind="ExternalOutput")

    with tile.TileContext(nc) as tc, ExitStack() as ctx:
        consts = ctx.enter_context(tc.tile_pool(name="consts", bufs=1))
        wpool = ctx.enter_context(tc.tile_pool(name="w", bufs=1))
        xt_pool = ctx.enter_context(tc.tile_pool(name="xt", bufs=8))
        qk_pool = ctx.enter_context(tc.tile_pool(name="qk", bufs=8))
        v_pool = ctx.enter_context(tc.tile_pool(name="v", bufs=8))
        ex_pool = ctx.enter_context(tc.tile_pool(name="ex", bufs=4))
        rt_pool = ctx.enter_context(tc.tile_pool(name="rt", bufs=8))
        misc = ctx.enter_context(tc.tile_pool(name="misc", bufs=2))
        # PSUM: 8 banks total. psb: 2 x [128,1024] = 4 banks (qk-proj, scores)
        # pss: tag res x4 x [128,512] = 4 banks (v-proj, AV accum, out-proj)
        psb = ctx.enter_context(tc.tile_pool(name="psb", bufs=2, space="PSUM"))
        pss = ctx.enter_context(tc.tile_pool(name="pss", bufs=4, space="PSUM"))

        ones = consts.tile([P, 64], AV_DT, tag="ones")
        nc.vector.memset(ones[:], 1.0)
        bqk_sb = consts.tile([P, 8], F32, tag="bqk")
        nc.sync.dma_start(out=bqk_sb[:], in_=bqk_d.ap().rearrange("(t p) -> p t", p=P))

        wqk_sb, wv_sb, wout_sb = [], [], []
        for kc in range(4):
            t = wpool.tile([P, 1024], MM_DT, tag=f"wqk{kc}")
            nc.sync.dma_start(out=t[:], in_=wqk_d.ap()[kc * P:(kc + 1) * P, :])
            wqk_sb.append(t)
        for kc in range(4):
            t = wpool.tile([P, 512], MM_DT, tag=f"wv{kc}")
            nc.sync.dma_start(out=t[:], in_=wv_d.ap()[kc * P:(kc + 1) * P, :])
            wv_sb.append(t)
            t = wpool.tile([P, 512], MM_DT, tag=f"wout{kc}")
            nc.sync.dma_start(out=t[:], in_=wout_d.ap()[kc * P:(kc + 1) * P, :])
            wout_sb.append(t)

        for b in [b for _ in range(repeat) for b in range(BL)]:
            # ---- load xT (channels on partitions) ----
            xt = []
            for kc in range(4):
                t = xt_pool.tile([P, N], MM_DT)
                nc.sync.dma_start(out=t[:], in_=xT_d.ap()[b, kc * P:(kc + 1) * P, :])
                xt.append(t)

            # ---- qk projection: qkT[j-tile] = w_qk_tile.T @ xT ----
            qk = []
            for jt in range(8):
                ps = psb.tile([P, N], F32, tag="big")
                for kc in range(4):
                    for xs in range(2):
                        nc.tensor.matmul(
                            ps[:, xs * 512:(xs + 1) * 512],
                            _mm(wqk_sb[kc][:, jt * P:(jt + 1) * P]),
                            _mm(xt[kc][:, xs * 512:(xs + 1) * 512]),
                            start=(kc == 0), stop=(kc == 3),
                        )
                t = qk_pool.tile([P, N], MM_DT)
                nc.vector.tensor_scalar(
                    out=t[:], in0=ps[:],
                    scalar1=bqk_sb[:, jt:jt + 1], scalar2=None,
                    op0=mybir.AluOpType.add,
                )
                qk.append(t)

            # ---- v projection (token-major) ----
            vo = []
            for yt in range(8):
                ps = pss.tile([P, 512], F32, tag="res")
                for kc in range(4):
                    nc.tensor.matmul(
                        ps[:],
                        _mm(xt[kc][:, yt * P:(yt + 1) * P]),
                        _mm(wv_sb[kc][:]),
                        start=(kc == 0), stop=(kc == 3),
                    )
                t = v_pool.tile([P, 512], AV_DT)
                nc.vector.tensor_copy(t[:], ps[:])
                vo.append(t)

            # ---- attention, head pairs ----
            rt = {}
            for p in range(4):
                kk = qk[p]
                qq = qk[4 + p]
                res_ps = [pss.tile([P, 512], F32, tag="res", name=f"res_{b}_{p}_{i}") for i in range(2)]
                s_ps = [pss.tile([P, 512], F32, tag="res", name=f"s_{b}_{p}_{i}") for i in range(2)]
                for z in (*res_ps, *s_ps):
                    nc.vector.memset(z[:], 0.0)
                def emit_av(yt, ex_pair):
                    last = yt == 7
                    for e in range(2):
                        h = 2 * p + e
                        for xs in range(2):
                            exs = ex_pair[e][:, xs * 512:(xs + 1) * 512]
                            nc.tensor.matmul(
                                res_ps[xs][e * 64:(e + 1) * 64, :],
                                vo[yt][:, h * 64:(h + 1) * 64],
                                exs,
                                start=False, stop=last, skip_group_check=True,
                            )
                            nc.tensor.matmul(
                                s_ps[xs][e * 64:(e + 1) * 64, :],
                                ones[:],
                                exs,
                                start=False, stop=last, skip_group_check=True,
                            )

                prev = None
                for yt in range(8):
                    ex = []
                    for e in range(2):
                        sc = psb.tile([P, N], F32, tag="big")
                        for xs in range(2):
                            nc.tensor.matmul(
                                sc[:, xs * 512:(xs + 1) * 512],
                                _mm(kk[e * 64:(e + 1) * 64, yt * P:(yt + 1) * P]),
                                _mm(qq[e * 64:(e + 1) * 64, xs * 512:(xs + 1) * 512]),
                                start=True, stop=True,
                            )
                        t = ex_pool.tile([P, N], AV_DT)
                        nc.scalar.activation(
                            out=t[:], in_=sc[:],
                            func=mybir.ActivationFunctionType.Exp, scale=SCALE,
                        )
                        ex.append(t)
                    if prev is not None:
                        emit_av(*prev)
                    prev = (yt, ex)
                emit_av(*prev)
                for xs in range(2):
                    rec = misc.tile([P, 512], F32, tag="prc")
                    nc.vector.reciprocal_approx_fast(out=rec[:], in_=s_ps[xs][:])
                    t = rt_pool.tile([P, 512], MM_DT)
                    nc.vector.tensor_tensor(
                        out=t[:], in0=res_ps[xs][:], in1=rec[:],
                        op=mybir.AluOpType.mult,
                    )
                    rt[(p, xs)] = t

            # ---- output projection + residual ----
            for nt in range(8):
                xs, sub = divmod(nt, 4)
                ps = pss.tile([P, 512], F32, tag="res")
                for p in range(4):
                    nc.tensor.matmul(
                        ps[:],
                        _mm(rt[(p, xs)][:, sub * P:(sub + 1) * P]),
                        _mm(wout_sb[p][:]),
                        start=(p == 0), stop=(p == 3),
                    )
                rs = misc.tile([P, 512], F32, tag="rs")
                nc.sync.dma_start(out=rs[:], in_=rs_d.ap()[b, nt * P:(nt + 1) * P, :])
                ob = misc.tile([P, 512], F32, tag="ob")
                nc.vector.tensor_tensor(
                    out=ob[:], in0=ps[:], in1=rs[:], op=mybir.AluOpType.add,
                )
                nc.sync.dma_start(out=out_d.ap()[b, nt * P:(nt + 1) * P, :], in_=ob[:])

    nc.compile()
    return nc


def host_prep(ft, w_qkv, b_qkv, w_out, b_out):
    ft = np.asarray(ft, dtype=np.float32)
    w_qkv = np.asarray(w_qkv, dtype=np.float32)
    b_qkv = np.asarray(b_qkv, dtype=np.float32)
    w_out = np.asarray(w_out, dtype=np.float32)
    b_out = np.asarray(b_out, dtype=np.float32)

    x = ft.reshape(B, N, C)
    xT = np.ascontiguousarray(x.transpose(0, 2, 1))

    w_qk_re = np.empty((C, 1024), np.float32)
    b_qk_re = np.empty((1024,), np.float32)
    w_v_re = np.empty((C, 512), np.float32)
    for p in range(4):
        hA, hB = 2 * p, 2 * p + 1
        w_qk_re[:, p * 128:p * 128 + 64] = w_qkv[:, hA * 192 + 64:hA * 192 + 128]
        w_qk_re[:, p * 128 + 64:p * 128 + 128] = w_qkv[:, hB * 192 + 64:hB * 192 + 128]
        b_qk_re[p * 128:p * 128 + 64] = b_qkv[hA * 192 + 64:hA * 192 + 128]
        b_qk_re[p * 128 + 64:p * 128 + 128] = b_qkv[hB * 192 + 64:hB * 192 + 128]
        w_qk_re[:, 512 + p * 128:512 + p * 128 + 64] = w_qkv[:, hA * 192:hA * 192 + 64]
        w_qk_re[:, 512 + p * 128 + 64:512 + p * 128 + 128] = w_qkv[:, hB * 192:hB * 192 + 64]
        b_qk_re[512 + p * 128:512 + p * 128 + 64] = b_qkv[hA * 192:hA * 192 + 64]
        b_qk_re[512 + p * 128 + 64:512 + p * 128 + 128] = b_qkv[hB * 192:hB * 192 + 64]
        w_v_re[:, p * 128:p * 128 + 64] = w_qkv[:, hA * 192 + 128:hA * 192 + 192]
        w_v_re[:, p * 128 + 64:p * 128 + 128] = w_qkv[:, hB * 192 + 128:hB * 192 + 192]

    b_v_nat = np.empty((512,), np.float32)
    for h in range(NH):
        b_v_nat[h * 64:(h + 1) * 64] = b_qkv[h * 192 + 128:h * 192 + 192]
    resid = x + b_out[None, None, :] + (b_v_nat @ w_out)[None, None, :]
    resid = np.ascontiguousarray(resid, dtype=np.float32)
    return xT, resid, w_qk_re, b_qk_re, w_v_re, np.ascontiguousarray(w_out)


_NC_CACHE = {}


def get_program(repeat: int = 1) -> bass.Bass:
    if repeat not in _NC_CACHE:
        _NC_CACHE[repeat] = build_program(repeat)
    return _NC_CACHE[repeat]


def make_in_maps(ft, w_qkv, b_qkv, w_out, b_out):
    xT, resid, w_qk_re, b_qk_re, w_v_re, w_out_c = host_prep(
        ft, w_qkv, b_qkv, w_out, b_out)
    in_maps = []
    for core in range(N_CORES):
        sl = slice(core * BL, (core + 1) * BL)
        in_maps.append({
            "xT": np.ascontiguousarray(xT[sl]),
            "resid": np.ascontiguousarray(resid[sl]),
            "wqk": w_qk_re,
            "bqk": b_qk_re,
            "wv": w_v_re,
            "wout": w_out_c,
        })
    return in_maps


_RUNNER_CACHE = {}


def make_runner(repeat: int = 1):
    """Build (once) a persistent jitted executor for the bass program.

    Returns run(in_maps) -> list[dict[name, np.ndarray]] per core. Keeping
    the jitted callable alive means repeat calls skip lowering/compile and
    are pure dispatch+execute.
    """
    if repeat in _RUNNER_CACHE:
        return _RUNNER_CACHE[repeat]

    import jax
    from jax.experimental.shard_map import shard_map
    from jax.sharding import Mesh, PartitionSpec
    from concourse import mybir as _mb
    from concourse import bass2jax

    bass2jax.install_neuronx_cc_hook()
    nc = get_program(repeat)

    partition_name = nc.partition_id_tensor.name if nc.partition_id_tensor else None
    in_names, out_names, out_avals, zero_shapes = [], [], [], []
    for alloc in nc.m.functions[0].allocations:
        if not isinstance(alloc, _mb.MemoryLocationSet):
            continue
        name = alloc.memorylocations[0].name
        if alloc.kind == "ExternalInput":
            if name != partition_name:
                in_names.append(name)
        elif alloc.kind == "ExternalOutput":
            np_dt = _mb.dt.np(alloc.dtype)
            out_names.append(name)
            out_avals.append(jax.core.ShapedArray(tuple(alloc.tensor_shape), np_dt))
            zero_shapes.append((tuple(alloc.tensor_shape), np_dt))
    n_params = len(in_names)
    all_in_names = list(in_names) + list(out_names)
    if partition_name is not None:
        all_in_names.append(partition_name)

    def _body(*args):
        operands = list(args)
        if partition_name is not None:
            operands.append(bass2jax.partition_id_tensor())
        outs = bass2jax._bass_exec_p.bind(
            *operands,
            out_avals=tuple(out_avals),
            in_names=tuple(all_in_names),
            out_names=tuple(out_names),
            lowering_input_output_aliases=(),
            sim_require_finite=True,
            sim_require_nnan=True,
            nc=nc,
        )
        return tuple(outs)

    devices = jax.devices()[:N_CORES]
    mesh = Mesh(np.asarray(devices), ("core",))
    n_outs = len(out_names)
    sharded = jax.jit(
        shard_map(_body, mesh=mesh,
                  in_specs=(PartitionSpec("core"),) * (n_params + n_outs),
                  out_specs=(PartitionSpec("core"),) * n_outs,
                  check_rep=False),
        keep_unused=True,
    )

    def run(in_maps):
        concat_in = [
            np.concatenate([np.asarray(m[name]) for m in in_maps], axis=0)
            for name in in_names
        ]
        zeros = [np.zeros((N_CORES * s[0], *s[1:]), dt) for s, dt in zero_shapes]
        out_arrs = sharded(*concat_in, *zeros)
        return [
            {name: np.asarray(out_arrs[i]).reshape(N_CORES, *out_avals[i].shape)[c]
             for i, name in enumerate(out_names)}
            for c in range(N_CORES)
        ]

    run.sharded = sharded
    run.in_names = in_names
    run.zero_shapes = zero_shapes
    run.mesh = mesh
    _RUNNER_CACHE[repeat] = run
    return run


def kernel(ft, w_qkv, b_qkv, w_out, b_out):
    run = make_runner()
    in_maps = make_in_maps(ft, w_qkv, b_qkv, w_out, b_out)
    results = run(in_maps)
    out = np.concatenate([r["out"] for r in results], axis=0)
    return out.reshape(B, HH, WW, C).astype(np.float32)
